# revision 33
# baseline (speedup 1.0000x reference)
"""Bass/Tile TRN2 kernel for retrieval-KNN MSE distance matrix.

Computes: out = ||t||^2 + ||s@W.T+b||^2 - 2 * t @ (s@W.T+b).T   [N=4096, M=4096]

Sharding (8 cores, output column-parallel, no collectives):
  core c holds s_rep rows [c*512, (c+1)*512) and computes the full-height
  output block out[:, c*512:(c+1)*512].  Per-core work:
    GEMM1: s_projT[1536, 512] = WT.T @ sT       (bf16, 12x12 k/j chunks)
    GEMM2: psum[128, 512] = (-2 t) @ s_projT    (fp8e4 DoubleRow, 6 K=256 MMs)
           + one bf16 K=1 "fold" matmul adding s_sq
  t_sq enters as the per-partition ACT/DVE bias at the PSUM drain, so the
  drain produces the final output value.

Host staging: t is shipped twice -- as fp8e4 pairs (d-major, pre-scaled by
-2) for the DoubleRow GEMM, and as row-major bf16 for t_sq, which is one
fused DVE tensor_tensor_reduce (square + row-sum -> [128,1]) per j-chunk.
s, W ship as bf16.  s_sq comes from a ones-matmul over squared bf16 s_proj.
"""

import numpy as np
import ml_dtypes

import concourse.bacc as bacc
import concourse.bass as bass
import concourse.mybir as mybir
import concourse.tile as tile
from concourse.bass_utils import run_bass_kernel_spmd

N = 4096          # t_rep rows
M = 4096          # s_rep rows
D = 1536          # feature dim
NCORES = 8
MC = M // NCORES  # 512: output columns per core
KC = D // 128     # 12:  contraction chunks
KP = KC // 2      # 6:   fp8 DoubleRow k-pairs
NJ = N // 128     # 32:  output row chunks per core
NG = N // 512     # 8:   512-row groups
WP = D // 512     # 3:   W column pieces

FP32 = mybir.dt.float32
BF16 = mybir.dt.bfloat16
FP8 = mybir.dt.float8e4
AF = mybir.ActivationFunctionType
ALU = mybir.AluOpType
BF16NP = ml_dtypes.bfloat16
FP8NP = ml_dtypes.float8_e4m3

N_WARM = 30


def build_nc(variant="full"):
    fp8_main = variant == "full"
    # fp8: psum holds -2*cross + s_sq, drain adds t_sq bias, scale 1.
    # bf16: psum holds cross - s_sq/2, drain scales by -2 and adds t_sq.
    fold_scale = 1.0 if fp8_main else -0.5
    nc = bacc.Bacc("TRN2", target_bir_lowering=False, num_devices=NCORES)

    t8_in = nc.dram_tensor("t8", [NG, 128, KP, 2, 512], FP8,
                           kind="ExternalInput").ap()
    tr_in = nc.dram_tensor("tr", [NG, 128, 4, D], FP8,
                           kind="ExternalInput").ap()
    s_in = nc.dram_tensor("s", [128, KP, 2, MC], FP8, kind="ExternalInput").ap()
    w_in = nc.dram_tensor("w", [KP, 128, 2, D], FP8,
                          kind="ExternalInput").ap()
    b_in = nc.dram_tensor("b", [128, KC], FP32, kind="ExternalInput").ap()
    out = nc.dram_tensor("out", [NJ // 2, 128, 2, MC], BF16,
                     kind="ExternalOutput").ap()

    with tile.TileContext(nc) as tc:
        with (
            tc.tile_pool(name="const", bufs=1) as const_pool,
            tc.tile_pool(name="sproj", bufs=1) as sproj_pool,
            tc.tile_pool(name="sprojf8", bufs=1) as sprojf8_pool,
            tc.tile_pool(name="small", bufs=2) as small_pool,
            tc.tile_pool(name="psum_main", bufs=4, space="PSUM") as psum_main,
        ):
            ones_col = const_pool.tile([128, 1], BF16)
            nc.vector.memset(ones_col[:], 1.0)
            # s_sq enters the main GEMM as a 7th DoubleRow pair, using the
            # two partition-0 lanes (c=0: 16*v0, c=1: v1 -- a 2-lane fp8
            # cascade of ssq, residual error < half an fp8 ulp of ~64).
            # Everything is built with partition-0 engine writes -- no DMAs
            # on this critical path.
            ssq_lhs = const_pool.tile([128, 2, 128], FP8)
            nc.vector.memset(ssq_lhs[:], 0.0)
            nc.vector.memset(ssq_lhs[0:1, 0, :], 16.0)
            nc.vector.memset(ssq_lhs[0:1, 1, :], 1.0)
            ssq_rhs = const_pool.tile([128, 2, MC], FP8)
            nc.vector.memset(ssq_rhs[:], 0.0)

            # ---- Phase 0: HAM warmup while initial DMAs stream ----
            with (
                tc.tile_pool(name="warmp", bufs=1) as warm_pool,
                tc.tile_pool(name="psum_warm", bufs=1, space="PSUM") as pw_pool,
            ):
                warm = warm_pool.tile([128, MC], BF16, name="warm")
                nc.vector.memset(warm[:], 0.5)
                pw = pw_pool.tile([128, MC], FP32, name="pw")
                for i in range(N_WARM):
                    nc.tensor.matmul(pw[:], lhsT=warm[:, 0:128], rhs=warm[:],
                                     start=(i == 0), stop=(i == N_WARM - 1))

            # ---- Phase 1: projection s_projT + s_sq; t groups stream in ----
            sproj = []    # 12 bf16 tiles [128, MC]
            sprojf8 = []  # 6 fp8 pair tiles [128, 2, MC]
            for p in range(KP):
                spf = sprojf8_pool.tile([128, 2, MC], FP8, name=f"sprojf8_{p}")
                sprojf8.append(spf)

            with (
                tc.tile_pool(name="wts", bufs=1) as wt_pool,
                tc.tile_pool(name="srep", bufs=1) as s_pool,
                tc.tile_pool(name="bias", bufs=1) as b_pool,
                tc.tile_pool(name="sq", bufs=KC) as sq_pool,
                tc.tile_pool(name="t8sb", bufs=3) as t8_pool,
                tc.tile_pool(name="trsb", bufs=3) as tr_pool,
                tc.tile_pool(name="ttrscratch", bufs=2) as ttr_pool,
                tc.tile_pool(name="tsqb", bufs=24) as tsq_pool,
                tc.tile_pool(name="osb", bufs=16) as out_pool,
                tc.tile_pool(name="psum_ssq", bufs=1, space="PSUM") as psum_ssq_pool,
            ):
                # -- DMA issue order: s, W pieces (c-major), b, early t groups --
                s8_sb = s_pool.tile([128, KP, 2, MC], FP8, name="s8_sb")
                nc.sync.dma_start(out=s8_sb[:], in_=s_in[:, :, :, :])
                w8_sb = []
                for p in range(KP):
                    w8 = wt_pool.tile([128, 2, D], FP8, name=f"w8_{p}")
                    nc.sync.dma_start(out=w8[:], in_=w_in[p])
                    w8_sb.append(w8)
                b_sb = b_pool.tile([128, KC], FP32, name="b_sb")
                nc.sync.dma_start(out=b_sb[:], in_=b_in[:, :])
                t8_sb, tr_sb = {}, {}
                for g in range(3):
                    t8 = t8_pool.tile([128, KP, 2, 512], FP8, name="t8")
                    nc.sync.dma_start(out=t8[:], in_=t8_in[g])
                    t8_sb[g] = t8
                    tr = tr_pool.tile([128, 4, D], FP8, name="tr")
                    nc.sync.dma_start(out=tr[:], in_=tr_in[g])
                    tr_sb[g] = tr

                # -- GEMM1: 12 j-blocks of 6 fp8 DoubleRow k-pair matmuls --
                psum_ssq = psum_ssq_pool.tile([1, MC], FP32, name="psum_ssq")
                sq_sb = []
                for j in range(KC):
                    ps = psum_main.tile([128, MC], FP32, name="psum_p1", tag="mm")
                    for p in range(KP):
                        nc.tensor.matmul(
                            ps[:],
                            lhsT=w8_sb[p][:, :, j * 128:(j + 1) * 128],
                            rhs=s8_sb[:, p, :, :],
                            start=(p == 0),
                            stop=(p == KP - 1),
                            perf_mode=mybir.MatmulPerfMode.DoubleRow,
                        )
                    sp = sproj_pool.tile([128, MC], BF16, name=f"sproj{j}")
                    nc.scalar.activation(sp[:], ps[:], AF.Identity,
                                         bias=b_sb[:, j:j + 1], scale=1.0)
                    sproj.append(sp)
                    if fp8_main:
                        # fp8 copy for the DoubleRow GEMM (DVE reads same psum)
                        nc.vector.tensor_scalar_add(
                            sprojf8[j // 2][:, j % 2, :], ps[:], b_sb[:, j:j + 1])
                    # squared projection for s_sq
                    sq = sq_pool.tile([128, MC], BF16, name="sq")
                    nc.vector.tensor_mul(sq[:], sp[:], sp[:])
                    sq_sb.append(sq)
                    # lag the s_sq ones-matmul four j-blocks so PE never waits
                    if j >= 4:
                        nc.tensor.matmul(psum_ssq[:], lhsT=ones_col[:],
                                         rhs=sq_sb[j - 4][:],
                                         start=(j == 4), stop=False)
                for j in (KC - 4, KC - 3, KC - 2, KC - 1):
                    nc.tensor.matmul(psum_ssq[:], lhsT=ones_col[:],
                                     rhs=sq_sb[j][:],
                                     start=False, stop=(j == KC - 1))
                # 2-lane fp8 cascade of s_sq at partition 0:
                #   lane (0,0): v0 = fp8(ssq/16), weighted 16 in ssq_lhs
                #   lane (0,1): v1 = fp8(ssq - 16*v0)
                ssq_f32 = small_pool.tile([1, MC], FP32, name="ssq_f32")
                nc.scalar.activation(ssq_f32[:], psum_ssq[:], AF.Identity,
                                     scale=fold_scale)
                nc.vector.tensor_scalar_mul(ssq_rhs[0:1, 0, :], ssq_f32[:],
                                            0.0625)
                r1 = small_pool.tile([1, MC], FP32, name="r1")
                nc.vector.scalar_tensor_tensor(
                    out=r1[:], in0=ssq_rhs[0:1, 0, :], scalar=-16.0,
                    in1=ssq_f32[:], op0=ALU.mult, op1=ALU.add)
                nc.vector.tensor_copy(ssq_rhs[0:1, 1, :], r1[:])

                # ---- Phase 2: main fp8 GEMM over 8 groups x 4 j-chunks ----
                pending_out = []
                for g in range(NG):
                    if g + 3 < NG:
                        t8 = t8_pool.tile([128, KP, 2, 512], FP8, name="t8")
                        nc.sync.dma_start(out=t8[:], in_=t8_in[g + 3])
                        t8_sb[g + 3] = t8
                        tr = tr_pool.tile([128, 4, D], FP8, name="tr")
                        nc.sync.dma_start(out=tr[:], in_=tr_in[g + 3])
                        tr_sb[g + 3] = tr
                    t8 = t8_sb.pop(g)
                    tr = tr_sb.pop(g)
                    # t_sq -> [128, 1] drain bias.  Two chunks on ScalarE
                    # (fused Square + row-sum accum_out), two on DVE via
                    # bn_stats/bn_aggr (one pass + tiny fixup).
                    tsq = []
                    for jj in range(4):
                        tb = tsq_pool.tile([128, 1], FP32, name="tb")
                        if jj < 2:
                            scr = ttr_pool.tile([128, D], BF16, name="scr")
                            nc.scalar.activation(scr[:], tr[:, jj, :], AF.Square,
                                                 accum_out=tb[:])
                        else:
                            bn6 = tsq_pool.tile([128, 3, 6], FP32, name="bn6")
                            for cc in range(3):
                                nc.vector.bn_stats(
                                    bn6[:, cc, :],
                                    tr[:, jj, cc * 512:(cc + 1) * 512])
                            bn2 = tsq_pool.tile([128, 2], FP32, name="bn2")
                            nc.vector.bn_aggr(bn2[:], bn6[:])
                            # t_sq = D * (var + mean^2)
                            msq = tsq_pool.tile([128, 1], FP32, name="msq")
                            nc.vector.tensor_mul(msq[:], bn2[:, 0:1], bn2[:, 0:1])
                            nc.vector.tensor_add(msq[:], msq[:], bn2[:, 1:2])
                            nc.vector.tensor_scalar_mul(tb[:], msq[:], float(D))
                        tsq.append(tb)

                    # flush deferred output stores (issued well after their
                    # drain so the sync stream never waits); drain the queue
                    # harder near the end so the tail doesn't serialize
                    while len(pending_out) > 1:
                        oj, oob = pending_out.pop(0)
                        nc.gpsimd.dma_start(out=out[oj], in_=oob[:])

                    for jj in range(4):
                        j = 4 * g + jj
                        if jj % 2 == 0:
                            ob = out_pool.tile([128, 2, MC], BF16, name="osb")
                        ps = psum_main.tile([128, MC], FP32, name="psum_main",
                                            tag="mm")
                        for p in range(KP):
                            nc.tensor.matmul(
                                ps[:],
                                lhsT=t8[:, p, :, jj * 128:(jj + 1) * 128],
                                rhs=sprojf8[p][:],
                                start=(p == 0),
                                stop=False,
                                perf_mode=mybir.MatmulPerfMode.DoubleRow,
                            )
                        # s_sq pair: += ssq[r] via the fp8 cascade lanes
                        nc.tensor.matmul(
                            ps[:],
                            lhsT=ssq_lhs[:],
                            rhs=ssq_rhs[:],
                            start=False,
                            stop=True,
                            perf_mode=mybir.MatmulPerfMode.DoubleRow,
                        )
                        if jj % 2 == 0:
                            nc.scalar.activation(ob[:, 0, :], ps[:], AF.Identity,
                                                 bias=tsq[jj][:], scale=1.0)
                        else:
                            nc.vector.tensor_scalar_add(ob[:, 1, :], ps[:],
                                                        tsq[jj][:])
                            pending_out.append((j // 2, ob))
                for (oj, oob) in pending_out:
                    nc.gpsimd.dma_start(out=out[oj], in_=oob[:])

    nc.compile()
    return nc


_NC_CACHE = {}


def _get_nc(variant="full"):
    if variant not in _NC_CACHE:
        _NC_CACHE[variant] = build_nc(variant)
    return _NC_CACHE[variant]


def stage_inputs(t_rep, s_rep, W, b):
    """Host-side layout staging (transpose/tile + dtype cast) -> per-core inputs."""
    t_rep = np.asarray(t_rep, dtype=np.float32)
    s_rep = np.asarray(s_rep, dtype=np.float32)
    W = np.asarray(W, dtype=np.float32)
    b = np.asarray(b, dtype=np.float32)

    # fp8 pairs, d-major, pre-scaled by -2:
    #   t8[g][p][P][c][r] = -2 * t[g*512+r, (2P+c)*128+p]
    t8 = np.ascontiguousarray(
        (-2.0 * t_rep).astype(FP8NP)
        .reshape(NG, 512, KP, 2, 128).transpose(0, 4, 2, 3, 1)
    )
    # row-major fp8: tr[g][p][jj][d] = t[g*512+jj*128+p, d]
    tr = np.ascontiguousarray(
        t_rep.astype(FP8NP).reshape(NG, 4, 128, D).transpose(0, 2, 1, 3)
    )
    # W fp8 pairs: w8[P][p, cc, m] = W[m, (2P+cc)*128+p]
    wt = np.ascontiguousarray(
        W.T.astype(FP8NP).reshape(KP, 2, 128, D).transpose(0, 2, 1, 3)
    )
    # b: [128, KC]; b_st[p, k] = b[k*128+p]
    b_st = np.ascontiguousarray(b.reshape(KC, 128).T)

    in_maps = []
    for c in range(NCORES):
        s_slice = s_rep[c * MC:(c + 1) * MC]  # [512, D]
        # fp8 pairs, partition-major: s8[p, P, cc, r] = s[r, (2P+cc)*128+p]
        s_st = np.ascontiguousarray(
            s_slice.astype(FP8NP).reshape(MC, KP, 2, 128).transpose(3, 1, 2, 0)
        )
        in_maps.append({"t8": t8, "tr": tr, "s": s_st, "w": wt, "b": b_st})
    return in_maps


def run_spmd(in_maps, variant="full", **kwargs):
    nc = _get_nc(variant)
    return run_bass_kernel_spmd(nc, in_maps, core_ids=list(range(NCORES)), **kwargs)


def gather_output(results):
    cols = []
    for c in range(NCORES):
        arr = results[c]["out"]  # [NJ//2, 128, 2, MC] bf16
        arr = np.transpose(arr, (0, 2, 1, 3)).reshape(N, MC)
        cols.append(arr.astype(np.float32))
    return np.concatenate(cols, axis=1)


def kernel(t_rep, s_rep, W, b):
    in_maps = stage_inputs(t_rep, s_rep, W, b)
    res = run_spmd(in_maps)
    return gather_output(res.results)
